# revision 3
# baseline (speedup 1.0000x reference)
import sys
sys.path.insert(0, '/opt/trn_rl_repo')
sys.path.insert(0, '/root/problem')
import numpy as np

_CACHE = {}


def kernel(h, W, bW, a1, a2, bA):
    from gat_build import build, host_inputs
    from concourse.bass_utils import run_bass_kernel_spmd

    B = h.shape[0]
    n_cores = 8
    nb = B // n_cores
    if "nc" not in _CACHE:
        _CACHE["nc"] = build(nb=nb, nh=W.shape[0], num_devices=n_cores)
    nc = _CACHE["nc"]
    in_maps = [host_inputs(h, W, bW, a1, a2, bA, slice(c * nb, (c + 1) * nb))
               for c in range(n_cores)]
    res = run_bass_kernel_spmd(nc, in_maps, core_ids=list(range(n_cores)))
    out = np.concatenate([res.results[c]["out"] for c in range(n_cores)], axis=0)
    return out


def run_traced(h, W, bW, a1, a2, bA, trace_cores=None, tmpdir=None):
    import prof_util; prof_util.enable()
    from gat_build import build, host_inputs
    from concourse.bass_utils import run_bass_kernel_spmd
    B = h.shape[0]
    n_cores = 8
    nb = B // n_cores
    if "nc" not in _CACHE:
        _CACHE["nc"] = build(nb=nb, nh=W.shape[0], num_devices=n_cores)
    nc = _CACHE["nc"]
    in_maps = [host_inputs(h, W, bW, a1, a2, bA, slice(c * nb, (c + 1) * nb))
               for c in range(n_cores)]
    res = run_bass_kernel_spmd(nc, in_maps, core_ids=list(range(n_cores)),
                               trace=True, trace_cores=trace_cores, tmpdir=tmpdir)
    return res


# revision 5
# speedup vs baseline: 1.0391x; 1.0391x over previous
import sys
sys.path.insert(0, '/opt/trn_rl_repo')
sys.path.insert(0, '/root/problem')
import numpy as np

_CACHE = {}


def kernel(h, W, bW, a1, a2, bA):
    from gat_build import build, host_inputs
    from concourse.bass_utils import run_bass_kernel_spmd

    B = h.shape[0]
    n_cores = 8
    nb = B // n_cores
    if "nc" not in _CACHE:
        _CACHE["nc"] = build(nb=nb, nh=W.shape[0], num_devices=n_cores)
    nc = _CACHE["nc"]
    in_maps = [host_inputs(h, W, bW, a1, a2, bA, slice(c * nb, (c + 1) * nb))
               for c in range(n_cores)]
    res = run_bass_kernel_spmd(nc, in_maps, core_ids=list(range(n_cores)))
    out = np.concatenate([res.results[c]["out"] for c in range(n_cores)], axis=0)
    return out


def run_traced(h, W, bW, a1, a2, bA, trace_cores=None, tmpdir=None):
    import prof_util; prof_util.enable()
    from gat_build import build, host_inputs
    from concourse.bass_utils import run_bass_kernel_spmd
    B = h.shape[0]
    n_cores = 8
    nb = B // n_cores
    if "nc" not in _CACHE:
        _CACHE["nc"] = build(nb=nb, nh=W.shape[0], num_devices=n_cores)
    nc = _CACHE["nc"]
    in_maps = [host_inputs(h, W, bW, a1, a2, bA, slice(c * nb, (c + 1) * nb))
               for c in range(n_cores)]
    res = run_bass_kernel_spmd(nc, in_maps, core_ids=list(range(n_cores)),
                               trace=True, trace_cores=trace_cores, tmpdir=tmpdir)
    return res
